# revision 12
# baseline (speedup 1.0000x reference)
"""Trainium2 Bass kernel for DMF embedding_lookup scoring.

reference computation:
    user_emb = (user_table[batch_users] + h_users_s) * 0.5        # [B, E]
    preds    = user_emb @ item_table.T                            # [B, N]
    returns (preds, user_emb, item_table)

Sharding: item_table (and the preds item dim) is split across the 8
NeuronCores; batch_users / h_users_s / user_table are replicated.  Each
core computes preds[:, shard] on device; the host concatenates shards.

Self-contained: only imports the environment-provided concourse package.
"""

import numpy as np

NUM_USERS = 100000
NUM_ITEMS = 100000
EMB = 64
BATCH = 2048
N_CORES = 8

# per-core shard, padded: 8 * 12544 = 100352 >= 100000
# 12544 = 128 * 98
ITEMS_PAD = 12544
N_TILES = 98          # 128-row item tiles per core
B_TILES = 16          # 2048 / 128
B_CHUNK = 512         # matmul moving free dim (PSUM bank limit for f32)
N_B_CHUNKS = 4        # 4 * 512 = 2048
TILE_PAIRS = 49       # item tiles are stored 2 per DMA (2 MB stores)

LAST_RESULTS = None  # BassKernelResults of the most recent run (for test.py)


def _build_bass():
    import concourse.bass as bass
    import concourse.bacc as bacc
    import concourse.mybir as mybir
    from concourse.masks import make_identity
    from concourse.tile import TileContext

    f32 = mybir.dt.float32
    f32r = mybir.dt.float32r
    # Bacc (not plain Bass): its finalize() runs generate_event_semaphores,
    # which splits multi-sem waits to the 1-wait-per-instruction HW limit.
    nc = bacc.Bacc("TRN2", target_bir_lowering=False, debug=False)

    item = nc.dram_tensor("item_shard", [ITEMS_PAD, EMB], f32, kind="ExternalInput")
    utab = nc.dram_tensor("user_table", [NUM_USERS, EMB], f32, kind="ExternalInput")
    hu = nc.dram_tensor("h_users", [BATCH, EMB], f32, kind="ExternalInput")
    idx = nc.dram_tensor("idx", [BATCH], mybir.dt.int32, kind="ExternalInput")
    # Transposed layout [items, batch]: items sit on PSUM partitions so the
    # item embeddings are the stationary matmul operand (one LDWEIGHTS per
    # 128-item tile, batch streams as the moving operand). Host transposes.
    preds = nc.dram_tensor(
        "preds_shard", [ITEMS_PAD, BATCH], f32, kind="ExternalOutput"
    )
    uemb = nc.dram_tensor("user_emb", [BATCH, EMB], f32, kind="ExternalOutput")

    with TileContext(nc) as tc:
        with (
            tc.tile_pool(name="const", bufs=1) as const_pool,
            tc.tile_pool(name="small", bufs=4) as small_pool,
            tc.tile_pool(name="ld", bufs=2) as ld_pool,
            tc.tile_pool(name="stg", bufs=3) as stg_pool,
            tc.tile_pool(name="psum_tr", bufs=2, space="PSUM") as psum_tr,
            tc.tile_pool(name="psum_mm", bufs=6, space="PSUM") as psum_mm,
        ):
            ident = const_pool.tile([128, 128], f32, tag="ident")
            make_identity(nc, ident[:])

            # ---- user path: gather + average + transpose ----
            # One indirect DMA for all 2048 indices and one bulk h load:
            # engine-sem waits merge by max tick, per-DMA sems do not, so
            # funneling through few DMA instructions keeps the per-inst
            # sync-wait count within the ISA limit.
            idxt = const_pool.tile([128, B_TILES], mybir.dt.int32, tag="idxt")
            nc.sync.dma_start(
                out=idxt[:], in_=idx[:].rearrange("(t p) -> p t", p=128)
            )
            # ---- item shard: load + transpose into resident itT [64, 12544] ----
            itT = const_pool.tile([64, ITEMS_PAD], f32r, tag="itT")
            item3 = item[:, :].rearrange("(n p) e -> p n e", p=128)
            groups = [25, 25, 25, 23]
            n0 = 0
            for gsz in groups:
                ld = ld_pool.tile([128, 25 * EMB], f32, tag="ld")
                nc.sync.dma_start(
                    out=ld[:, : gsz * EMB].rearrange("p (n e) -> p n e", e=EMB),
                    in_=item3[:, n0 : n0 + gsz, :],
                )
                for j in range(gsz):
                    n = n0 + j
                    ps = psum_tr.tile([64, 128], f32, tag="ps_tr")
                    nc.tensor.transpose(
                        out=ps[:],
                        in_=ld[:, j * EMB : (j + 1) * EMB],
                        identity=ident[:],
                    )
                    nc.vector.tensor_copy(
                        out=itT[:, n * 128 : (n + 1) * 128], in_=ps[:]
                    )
                n0 += gsz

            u_all = const_pool.tile([128, B_TILES * EMB], f32, tag="uall")
            h_all = const_pool.tile([128, B_TILES * EMB], f32, tag="hall")
            uT = const_pool.tile([64, BATCH], f32r, tag="uT")
            # One indirect DMA per 128-row batch tile ([128, 1] offsets — the
            # production-proven shape; multi-column offset APs gather wrong
            # data on HW even though CoreSim accepts them).
            for t in range(B_TILES):
                nc.gpsimd.indirect_dma_start(
                    out=u_all[:, t * EMB : (t + 1) * EMB],
                    out_offset=None,
                    in_=utab[:, :],
                    in_offset=bass.IndirectOffsetOnAxis(
                        ap=idxt[:, t : t + 1], axis=0
                    ),
                )
            nc.sync.dma_start(
                out=h_all[:].rearrange("p (t e) -> p t e", e=EMB),
                in_=hu[:, :].rearrange("(t p) e -> p t e", p=128),
            )
            nc.vector.tensor_add(out=u_all[:], in0=u_all[:], in1=h_all[:])
            nc.scalar.mul(out=u_all[:], in_=u_all[:], mul=0.5)
            for t in range(B_TILES):
                ps = psum_tr.tile([64, 128], f32, tag="ps_tr")
                nc.tensor.transpose(
                    out=ps[:],
                    in_=u_all[:, t * EMB : (t + 1) * EMB],
                    identity=ident[:],
                )
                nc.vector.tensor_copy(out=uT[:, t * 128 : (t + 1) * 128], in_=ps[:])
            nc.sync.dma_start(
                out=uemb[:, :].rearrange("(t p) e -> p t e", p=128),
                in_=u_all[:].rearrange("p (t e) -> p t e", e=EMB),
            )

            # ---- GEMM: preds_T[n*128:(n+1)*128, :] = itT[:, ntile].T @ uT ----
            # fp32r operands: single-pass PE matmul (1 cycle/row at free
            # dim >= 256) vs fp32's two half-speed passes (4 cycles/row).
            for pair in range(TILE_PAIRS):
                stg = stg_pool.tile([128, 2 * BATCH], f32, tag="stg")
                for s in range(2):
                    n = 2 * pair + s
                    for b in range(N_B_CHUNKS):
                        ps = psum_mm.tile([128, B_CHUNK], f32, tag="ps_mm")
                        nc.tensor.matmul(
                            out=ps[:],
                            lhsT=itT[:, n * 128 : (n + 1) * 128],
                            rhs=uT[:, b * B_CHUNK : (b + 1) * B_CHUNK],
                            start=True,
                            stop=True,
                        )
                        col0 = s * BATCH + b * B_CHUNK
                        if b % 2 == 0:
                            nc.vector.tensor_copy(
                                out=stg[:, col0 : col0 + B_CHUNK], in_=ps[:]
                            )
                        else:
                            nc.scalar.copy(
                                out=stg[:, col0 : col0 + B_CHUNK], in_=ps[:]
                            )
                nc.sync.dma_start(
                    out=preds[pair * 256 : (pair + 1) * 256, :].rearrange(
                        "(s p) b -> p s b", p=128
                    ),
                    in_=stg[:].rearrange("p (s b) -> p s b", b=BATCH),
                )
    # run_bass_via_pjrt does not finalize; Bacc.finalize() runs the bacc
    # compile pipeline (register alloc, event-semaphore wait splitting).
    nc.finalize()
    return nc


def kernel(batch_users, h_users_s, user_table, item_table):
    global LAST_RESULTS
    from concourse.bass_utils import run_bass_kernel_spmd

    item_table_in = item_table
    batch_users = np.ascontiguousarray(np.asarray(batch_users).astype(np.int32))
    h_users_s = np.ascontiguousarray(np.asarray(h_users_s, dtype=np.float32))
    user_table = np.ascontiguousarray(np.asarray(user_table, dtype=np.float32))
    item_f32 = np.ascontiguousarray(np.asarray(item_table, dtype=np.float32))

    item_pad = np.zeros((N_CORES * ITEMS_PAD, EMB), np.float32)
    item_pad[:NUM_ITEMS] = item_f32

    nc = _build_bass()
    in_maps = [
        {
            "item_shard": np.ascontiguousarray(
                item_pad[i * ITEMS_PAD : (i + 1) * ITEMS_PAD]
            ),
            "user_table": user_table,
            "h_users": h_users_s,
            "idx": batch_users,
        }
        for i in range(N_CORES)
    ]
    res = run_bass_kernel_spmd(nc, in_maps, core_ids=list(range(N_CORES)))
    LAST_RESULTS = res

    preds_t = np.concatenate(
        [r["preds_shard"] for r in res.results], axis=0
    )[:NUM_ITEMS]
    preds = np.ascontiguousarray(preds_t.T)
    user_emb = res.results[0]["user_emb"]
    return preds, user_emb, item_table_in


# revision 14
# speedup vs baseline: 1.2267x; 1.2267x over previous
"""Trainium2 Bass kernel for DMF embedding_lookup scoring.

reference computation:
    user_emb = (user_table[batch_users] + h_users_s) * 0.5        # [B, E]
    preds    = user_emb @ item_table.T                            # [B, N]
    returns (preds, user_emb, item_table)

Sharding: item_table (and the preds item dim) is split across the 8
NeuronCores; batch_users / h_users_s / user_table are replicated.  Each
core computes preds[:, shard] on device; the host concatenates shards.

Self-contained: only imports the environment-provided concourse package.
"""

import numpy as np

NUM_USERS = 100000
NUM_ITEMS = 100000
EMB = 64
BATCH = 2048
N_CORES = 8

# per-core shard, padded: 8 * 12544 = 100352 >= 100000
# 12544 = 128 * 98
ITEMS_PAD = 12544
N_TILES = 98          # 128-row item tiles per core
B_TILES = 16          # 2048 / 128
B_CHUNK = 512         # matmul moving free dim (PSUM bank limit for f32)
N_B_CHUNKS = 4        # 4 * 512 = 2048
TILE_PAIRS = 49       # item tiles are stored 2 per DMA (2 MB stores)

LAST_RESULTS = None  # BassKernelResults of the most recent run (for test.py)


def _build_bass():
    import concourse.bass as bass
    import concourse.bacc as bacc
    import concourse.mybir as mybir
    from concourse.masks import make_identity
    from concourse.tile import TileContext

    f32 = mybir.dt.float32
    f32r = mybir.dt.float32r
    # Bacc (not plain Bass): its finalize() runs generate_event_semaphores,
    # which splits multi-sem waits to the 1-wait-per-instruction HW limit.
    nc = bacc.Bacc("TRN2", target_bir_lowering=False, debug=False)

    item = nc.dram_tensor("item_shard", [ITEMS_PAD, EMB], f32, kind="ExternalInput")
    utab = nc.dram_tensor("user_table", [NUM_USERS, EMB], f32, kind="ExternalInput")
    hu = nc.dram_tensor("h_users", [BATCH, EMB], f32, kind="ExternalInput")
    idx = nc.dram_tensor("idx", [BATCH], mybir.dt.int32, kind="ExternalInput")
    # Transposed layout [items, batch]: items sit on PSUM partitions so the
    # item embeddings are the stationary matmul operand (one LDWEIGHTS per
    # 128-item tile, batch streams as the moving operand). Host transposes.
    preds = nc.dram_tensor(
        "preds_shard", [ITEMS_PAD, BATCH], f32, kind="ExternalOutput"
    )
    uemb = nc.dram_tensor("user_emb", [BATCH, EMB], f32, kind="ExternalOutput")

    with TileContext(nc) as tc:
        with (
            tc.tile_pool(name="const", bufs=1) as const_pool,
            tc.tile_pool(name="small", bufs=4) as small_pool,
            tc.tile_pool(name="ld", bufs=2) as ld_pool,
            tc.tile_pool(name="stg", bufs=3) as stg_pool,
            tc.tile_pool(name="psum_tr", bufs=3, space="PSUM") as psum_tr,
            tc.tile_pool(name="psum_mm", bufs=5, space="PSUM") as psum_mm,
        ):
            ident = const_pool.tile([128, 128], f32, tag="ident")
            make_identity(nc, ident[:])

            # ---- user path: gather + average + transpose ----
            # One indirect DMA for all 2048 indices and one bulk h load:
            # engine-sem waits merge by max tick, per-DMA sems do not, so
            # funneling through few DMA instructions keeps the per-inst
            # sync-wait count within the ISA limit.
            idxt = const_pool.tile([128, B_TILES], mybir.dt.int32, tag="idxt")
            nc.sync.dma_start(
                out=idxt[:], in_=idx[:].rearrange("(t p) -> p t", p=128)
            )
            # ---- item shard: load + transpose into resident itT ----
            # Item tiles are transposed two at a time: one [128, 128] PE
            # transpose leaves tile 2k on partitions 0-63 and tile 2k+1 on
            # partitions 64-127, and one full-width cast copies both into
            # itT (split-partition layout). Odd tiles feed matmul as lhsT
            # directly from partitions 64-127.
            itT = const_pool.tile([128, TILE_PAIRS * 128], f32r, tag="itT")
            item3 = item[:, :].rearrange("(n p) e -> p n e", p=128)
            groups = [24, 24, 24, 26]
            n0 = 0
            for gsz in groups:
                ld = ld_pool.tile([128, 26 * EMB], f32, tag="ld")
                nc.sync.dma_start(
                    out=ld[:, : gsz * EMB].rearrange("p (n e) -> p n e", e=EMB),
                    in_=item3[:, n0 : n0 + gsz, :],
                )
                for j in range(0, gsz, 2):
                    k = (n0 + j) // 2
                    ps = psum_tr.tile([128, 128], f32, tag="ps_tr")
                    nc.tensor.transpose(
                        out=ps[:],
                        in_=ld[:, j * EMB : (j + 2) * EMB],
                        identity=ident[:],
                    )
                    nc.vector.tensor_copy(
                        out=itT[:, k * 128 : (k + 1) * 128], in_=ps[:]
                    )
                n0 += gsz

            u_all = const_pool.tile([128, B_TILES * EMB], f32, tag="uall")
            h_all = const_pool.tile([128, B_TILES * EMB], f32, tag="hall")
            # uT replicated on partitions 0-63 and 64-127: matmul requires
            # lhsT and rhs at the same base partition, and odd item tiles
            # read lhsT from the upper half (split-partition itT layout).
            uT = const_pool.tile([128, BATCH], f32r, tag="uT")
            # One indirect DMA per 128-row batch tile ([128, 1] offsets — the
            # production-proven shape; multi-column offset APs gather wrong
            # data on HW even though CoreSim accepts them).
            for t in range(B_TILES):
                nc.gpsimd.indirect_dma_start(
                    out=u_all[:, t * EMB : (t + 1) * EMB],
                    out_offset=None,
                    in_=utab[:, :],
                    in_offset=bass.IndirectOffsetOnAxis(
                        ap=idxt[:, t : t + 1], axis=0
                    ),
                )
            nc.sync.dma_start(
                out=h_all[:].rearrange("p (t e) -> p t e", e=EMB),
                in_=hu[:, :].rearrange("(t p) e -> p t e", p=128),
            )
            nc.vector.tensor_add(out=u_all[:], in0=u_all[:], in1=h_all[:])
            nc.scalar.mul(out=u_all[:], in_=u_all[:], mul=0.5)
            for t in range(B_TILES):
                ps = psum_tr.tile([64, 128], f32, tag="ps_tr")
                nc.tensor.transpose(
                    out=ps[:],
                    in_=u_all[:, t * EMB : (t + 1) * EMB],
                    identity=ident[:],
                )
                nc.vector.tensor_copy(
                    out=uT[0:64, t * 128 : (t + 1) * 128], in_=ps[:]
                )
            nc.sync.dma_start(
                out=uemb[:, :].rearrange("(t p) e -> p t e", p=128),
                in_=u_all[:].rearrange("p (t e) -> p t e", e=EMB),
            )
            nc.sync.dma_start(out=uT[64:128, :], in_=uT[0:64, :])

            # ---- GEMM: preds_T[n*128:(n+1)*128, :] = itT[:, ntile].T @ uT ----
            # fp32r operands: single-pass PE matmul (1 cycle/row at free
            # dim >= 256) vs fp32's two half-speed passes (4 cycles/row).
            for pair in range(TILE_PAIRS):
                stg = stg_pool.tile([128, 2 * BATCH], f32, tag="stg")
                for s in range(2):
                    p0 = 64 * s
                    for b in range(N_B_CHUNKS):
                        ps = psum_mm.tile([128, B_CHUNK], f32, tag="ps_mm")
                        nc.tensor.matmul(
                            out=ps[:],
                            lhsT=itT[p0 : p0 + 64, pair * 128 : (pair + 1) * 128],
                            rhs=uT[
                                p0 : p0 + 64, b * B_CHUNK : (b + 1) * B_CHUNK
                            ],
                            start=True,
                            stop=True,
                        )
                        col0 = s * BATCH + b * B_CHUNK
                        if b % 2 == 0:
                            nc.vector.tensor_copy(
                                out=stg[:, col0 : col0 + B_CHUNK], in_=ps[:]
                            )
                        else:
                            nc.scalar.copy(
                                out=stg[:, col0 : col0 + B_CHUNK], in_=ps[:]
                            )
                nc.sync.dma_start(
                    out=preds[pair * 256 : (pair + 1) * 256, :].rearrange(
                        "(s p) b -> p s b", p=128
                    ),
                    in_=stg[:].rearrange("p (s b) -> p s b", b=BATCH),
                )
    # run_bass_via_pjrt does not finalize; Bacc.finalize() runs the bacc
    # compile pipeline (register alloc, event-semaphore wait splitting).
    nc.finalize()
    return nc


def kernel(batch_users, h_users_s, user_table, item_table):
    global LAST_RESULTS
    from concourse.bass_utils import run_bass_kernel_spmd

    item_table_in = item_table
    batch_users = np.ascontiguousarray(np.asarray(batch_users).astype(np.int32))
    h_users_s = np.ascontiguousarray(np.asarray(h_users_s, dtype=np.float32))
    user_table = np.ascontiguousarray(np.asarray(user_table, dtype=np.float32))
    item_f32 = np.ascontiguousarray(np.asarray(item_table, dtype=np.float32))

    item_pad = np.zeros((N_CORES * ITEMS_PAD, EMB), np.float32)
    item_pad[:NUM_ITEMS] = item_f32

    nc = _build_bass()
    in_maps = [
        {
            "item_shard": np.ascontiguousarray(
                item_pad[i * ITEMS_PAD : (i + 1) * ITEMS_PAD]
            ),
            "user_table": user_table,
            "h_users": h_users_s,
            "idx": batch_users,
        }
        for i in range(N_CORES)
    ]
    res = run_bass_kernel_spmd(nc, in_maps, core_ids=list(range(N_CORES)))
    LAST_RESULTS = res

    preds_t = np.concatenate(
        [r["preds_shard"] for r in res.results], axis=0
    )[:NUM_ITEMS]
    preds = np.ascontiguousarray(preds_t.T)
    user_emb = res.results[0]["user_emb"]
    return preds, user_emb, item_table_in
